# revision 28
# baseline (speedup 1.0000x reference)
"""Trainium2 Bass kernel for nn_GroupLocalSL2 (grouped gather + conv).

out[b,o,i,xo,yo] = sum_{c,f,kh,kw} x[b,c,idx[i,f],xo+kh,yo+kw] * W[o,c,f,kh,kw] + bias[o]

Strategy:
  - Batch B=8 sharded across 8 NeuronCores (data parallel), one b per core.
  - Per core, per output group i: gather 15 row-shifted copies of the G_F=7
    selected images into 4 SBUF tiles so that every matmul runs with a fully
    packed K=128 contraction (tap (f,kh) covered by tile block (f,s) at instr
    row-offset D, kh = s + D):
      T1 = (f0@0,f1@0,f2@0,f3@0)  used at D=0..4   -> 20 taps
      T2 = (f4@0,f4@1,f5@0,f5@1)  used at D in {0,2} -> 8 taps
      T3 = (f6@0,f6@1,f6@2,f6@3)  used at D=0      -> 4 taps
      T4 = (f6@4,f4@4,f5@4)       used at D=0      -> 3 taps (K=96)
  - kw0..3 accumulate into ONE psum block via column-shifted rhs: colset 0
    (kw0,kw1) at col offset 0, colset 1 (kw2,kw3) at col offset 2; psum class
    a holds (kw_a + kw_{a+2}) aligned at out col c (class 0) / c-1 (class 1).
    18 matmuls, N = R*61.
  - kw4: two concurrent M=64 column-tiled accumulation chains (psum halves),
    9 matmuls in 5 serial slots, N = R*60.
  - Compute in bf16 (host casts x/W), fp32 PSUM accumulate; rel err ~2e-3.
  - Combine: 1 ScalarE (bias) + 3 VectorE adds, DMA to DRAM.
"""

import os
import sys

import numpy as np
import ml_dtypes

for _p in ("/opt/trn_rl_repo", "/root/.axon_site/_ro/trn_rl_repo"):
    if os.path.isdir(_p) and _p not in sys.path:
        sys.path.append(_p)

import concourse.bass as bass
import concourse.mybir as mybir
import concourse.tile as tile
from concourse import bacc
from concourse.bass_utils import run_bass_kernel_spmd

BF16 = ml_dtypes.bfloat16

B, C, G_IN = 8, 32, 33
O, G_F, KH, KW = 64, 7, 5, 5
X, Y = 64, 64
G_OUT = 15
XO, YO = X - KH + 1, Y - KW + 1  # 60, 60
RCH = 8  # output rows per chunk (8*61 = 488 <= 512 psum bank)

# Instruction list: (tile_id, delta). Tile block layout: list of (f, s).
TILE_BLOCKS = {
    1: [(0, 0), (1, 0), (2, 0), (3, 0)],
    2: [(4, 0), (4, 1), (5, 0), (5, 1)],
    3: [(6, 0), (6, 1), (6, 2), (6, 3)],
    4: [(6, 4), (4, 4), (5, 4)],
}
# Tile row counts (rows materialized in SBUF per block)
TILE_ROWS = {1: 64, 2: 62, 3: 60, 4: 60}
INSTRS = (
    [(1, d) for d in range(5)]
    + [(2, 0), (2, 2)]
    + [(3, 0)]
    + [(4, 0)]
)  # 9 instrs; last has K=96


def _build_nc(idx, n_groups=G_OUT):
    """Build the single-core Bass program (idx values baked into DMAs)."""
    nc = bacc.Bacc("TRN2", target_bir_lowering=False, debug=False)
    dt = mybir.dt
    xin = nc.dram_tensor("x", [C, G_IN, X, Y], dt.bfloat16, kind="ExternalInput")
    wp_d = nc.dram_tensor("wp", [128, 9, 256], dt.bfloat16, kind="ExternalInput")
    wq_d = nc.dram_tensor("wq", [128, 9, O], dt.bfloat16, kind="ExternalInput")
    bias_d = nc.dram_tensor("bias", [O, 1], dt.float32, kind="ExternalInput")
    out_d = nc.dram_tensor("out", [O, G_OUT, XO, YO], dt.float32, kind="ExternalOutput")

    rchunks = [(r0, min(RCH, XO - r0)) for r0 in range(0, XO, RCH)]

    with tile.TileContext(nc) as tc:
        with (
            tc.tile_pool(name="wpool", bufs=1) as wpool,
            tc.tile_pool(name="xpool", bufs=3) as xpool,
            tc.tile_pool(name="tpool", bufs=3) as tpool,
            tc.tile_pool(name="opool", bufs=4) as opool,
            tc.tile_pool(name="psum", bufs=4, space="PSUM") as pp,
            tc.tile_pool(name="psum2", bufs=4, space="PSUM") as pp2,
        ):
            # PE clock warm-up: the first matmuls after idle run at the cold
            # ~1.2GHz rate (~406ns vs 208ns for N=488). Ramp the clock with
            # tiny dummy matmuls on memset data before the real work lands.
            warm = wpool.tile([1, 512], dt.bfloat16, tag="warm")
            pwarm = pp.tile([128, RCH, 61], dt.float32, tag="pP", name="pwarm")
            nc.gpsimd.memset(warm[:, :], 0.0)
            for _ in range(34):
                nc.tensor.matmul(
                    pwarm[0:1, :, :], warm[0:1, 0:1], warm[0:1, 0:488],
                    start=True, stop=True,
                )

            wp = wpool.tile([128, 9, 256], dt.bfloat16, tag="wp")
            wq = wpool.tile([128, 9, O], dt.bfloat16, tag="wq")
            bias_sb = wpool.tile([O, 1], dt.float32, tag="bias")
            # weights on the Activation DGE: parallel with x gathers (sync
            # DGE). Split wp so the T1-instr weights (q=0..4) land first —
            # the head's first matmuls need only those plus t1.
            nc.scalar.dma_start(wp[:, 0:5, :], wp_d[:, 0:5, :])
            nc.scalar.dma_start(wp[:, 5:9, :], wp_d[:, 5:9, :])
            nc.scalar.dma_start(wq[:, :, :], wq_d[:, :, :])
            nc.scalar.dma_start(bias_sb[:, :], bias_d[:, :])

            for i in range(n_groups):
                # Gather the 15 row-shifted image copies into 4 tiles.
                # t4 first: the tile scheduler models DMA arrival from issue
                # order; a late-modeled t4 makes it defer T4 matmuls, which
                # splits the M64 blocks and adds PE mode-switch stalls.
                tiles = {}
                for tid in (1, 2, 3, 4) if i == 0 else (4, 1, 2, 3):
                    nr = TILE_ROWS[tid]
                    blocks = TILE_BLOCKS[tid]
                    npart = 32 * len(blocks)
                    t = xpool.tile([npart, nr, Y], dt.bfloat16, tag=f"t{tid}")
                    tiles[tid] = t
                    for b, (f, s) in enumerate(blocks):
                        g = int(idx[i, f])
                        nc.sync.dma_start(
                            t[b * 32 : (b + 1) * 32, :, :],
                            xin[:, g, s : s + nr, :],
                        )

                # kw0..3: 18 matmuls per rchunk accumulating into pP.
                # colset cs reads x cols 2cs..2cs+60; lhsT cols
                # [cs*128 + a*64 + o] hold w[.., kw=2cs+a].
                def mm_p(pP, r0, R, cs, q, start, stop):
                    tid, d = INSTRS[q]
                    xt = tiles[tid]
                    Kq = 32 * len(TILE_BLOCKS[tid])
                    nc.tensor.matmul(
                        pP[:, 0:R, :],
                        wp[0:Kq, q, cs * 128 : cs * 128 + 128],
                        xt[0:Kq, r0 + d : r0 + d + R, 2 * cs : 2 * cs + 61],
                        start=start,
                        stop=stop,
                    )

                # kw4: two concurrent M=64 column-tiled chains.
                def mm_q2(p2, r0, R, q, half, start, stop):
                    tid, d = INSTRS[q]
                    xt = tiles[tid]
                    Kq = 32 * len(TILE_BLOCKS[tid])
                    nc.tensor.matmul(
                        p2[half * 64 : half * 64 + 64, 0:R, :],
                        wq[0:Kq, q, :],
                        xt[0:Kq, r0 + d : r0 + d + R, 4 : 4 + 60],
                        start=start,
                        stop=stop,
                    )

                # Process rchunks in waves of 4: all M128 pP work, then all
                # M64 kw4 work — quarters PE full/column-tiled mode switches.
                for w0 in range(0, len(rchunks), 4):
                    wave = rchunks[w0 : w0 + 4]
                    pPs, p2s = [], []
                    for r0, R in wave:
                        pPs.append(
                            pp.tile([128, RCH, 61], dt.float32, tag="pP", name="pP")
                        )
                        p2s.append(
                            pp2.tile([128, RCH, 60], dt.float32, tag="p2", name="p2")
                        )
                    if i == 0 and w0 == 0:
                        # head: run every rchunk's T1-only prefix first —
                        # those need just t1 + the first wp slice, giving
                        # the PE a runway while t2..t4 stream in.
                        for (r0, R), pP in zip(wave, pPs):
                            for q in range(5):
                                mm_p(pP, r0, R, 0, q, start=(q == 0), stop=False)
                        for (r0, R), pP in zip(wave, pPs):
                            for q in range(5, 9):
                                mm_p(pP, r0, R, 0, q, start=False, stop=False)
                            for q in range(9):
                                mm_p(pP, r0, R, 1, q, start=False, stop=(q == 8))
                    else:
                        for (r0, R), pP in zip(wave, pPs):
                            for cs in range(2):
                                for q in range(9):
                                    mm_p(
                                        pP, r0, R, cs, q,
                                        start=(cs == 0 and q == 0),
                                        stop=(cs == 1 and q == 8),
                                    )
                    # kw4 column-tile pairs must be CROSS-tile (same-tile
                    # concurrent streams contend on SBUF reads): h0 runs
                    # T1@0..4, h1 runs T2/T3/T4, interleaved pairwise.
                    for (r0, R), p2 in zip(wave, p2s):
                        for k, (q, start, stop) in enumerate(
                            [
                                (0, True, False),   # T1@0  h0 start
                                (5, True, False),   # T2@0  h1 start
                                (1, False, False),  # T1@1  h0
                                (6, False, False),  # T2@2  h1
                                (2, False, False),  # T1@2  h0
                                (7, False, False),  # T3@0  h1
                                (3, False, False),  # T1@3  h0
                                (8, False, True),   # T4@0  h1 stop
                                (4, False, True),   # T1@4  h0 stop
                            ]
                        ):
                            mm_q2(p2, r0, R, q, half=k % 2, start=start, stop=stop)

                    # Combine: at most ONE PSUM operand per instruction.
                    # out[c] = bias + pP[0:64,:,c] + pP[64:,:,c+1]
                    #        + p2[0:64,:,c] + p2[64:,:,c]
                    for (r0, R), pP, p2 in zip(wave, pPs, p2s):
                        t = tpool.tile([O, RCH, 60], dt.float32, tag="t")
                        ot = opool.tile([O, RCH, 60], dt.float32, tag="out")
                        nc.scalar.add(
                            t[:, 0:R, :], pP[0:64, 0:R, 0:60], bias_sb[:, 0:1]
                        )
                        nc.vector.tensor_add(
                            t[:, 0:R, :], t[:, 0:R, :], pP[64:128, 0:R, 1:61]
                        )
                        nc.vector.tensor_add(
                            t[:, 0:R, :], t[:, 0:R, :], p2[0:64, 0:R, :]
                        )
                        nc.vector.tensor_add(
                            ot[:, 0:R, :], t[:, 0:R, :], p2[64:128, 0:R, :]
                        )
                        # out via the Activation-engine DGE: separate DMA
                        # queues from the x gathers, so gathers never sit
                        # behind compute-gated output writes in a queue FIFO.
                        nc.scalar.dma_start(
                            out_d[:, i, r0 : r0 + R, :], ot[:, 0:R, :]
                        )
    nc.compile()
    return nc


def _prep_weights(weight, bias):
    """Host-side lhsT weight layout for the 9-instruction schedule."""
    w = np.asarray(weight).astype(np.float32)  # [O, C, G_F, KH, KW]
    wp = np.zeros((128, 9, 256), dtype=np.float32)
    wq = np.zeros((128, 9, O), dtype=np.float32)
    for q, (tid, d) in enumerate(INSTRS):
        for b, (f, s) in enumerate(TILE_BLOCKS[tid]):
            kh = s + d
            rows = slice(b * 32, (b + 1) * 32)
            for cs in range(2):
                for a in range(2):
                    kw = 2 * cs + a
                    col = cs * 128 + a * 64
                    # wp[row=c, q, col+o] = w[o, c, f, kh, kw]
                    wp[rows, q, col : col + O] = w[:, :, f, kh, kw].T
            wq[rows, q, :] = w[:, :, f, kh, 4].T
    b2 = np.ascontiguousarray(np.asarray(bias).astype(np.float32).reshape(O, 1))
    return wp.astype(BF16), wq.astype(BF16), b2


def _prep_inputs(x, weight, bias, idx):
    x16 = np.asarray(x).astype(BF16)  # [B, C, G_IN, X, Y]
    wp, wq, b2 = _prep_weights(weight, bias)
    in_maps = []
    for b in range(B):
        in_maps.append(
            {
                "x": np.ascontiguousarray(x16[b]),
                "wp": wp,
                "wq": wq,
                "bias": b2,
            }
        )
    return in_maps


def run(x, weight, bias, idx, trace=False):
    idx = np.asarray(idx).astype(np.int64)
    assert idx.shape == (G_OUT, G_F) and idx.min() >= 0 and idx.max() < G_IN
    nc = _build_nc(idx)
    in_maps = _prep_inputs(x, weight, bias, idx)
    res = run_bass_kernel_spmd(nc, in_maps, list(range(B)), trace=trace)
    out = np.stack([res.results[b]["out"] for b in range(B)]).astype(np.float32)
    return out, res


def kernel(x, weight, bias, idx):
    out, _ = run(x, weight, bias, idx, trace=False)
    return out


# revision 29
# speedup vs baseline: 1.0066x; 1.0066x over previous
"""Trainium2 Bass kernel for nn_GroupLocalSL2 (grouped gather + conv).

out[b,o,i,xo,yo] = sum_{c,f,kh,kw} x[b,c,idx[i,f],xo+kh,yo+kw] * W[o,c,f,kh,kw] + bias[o]

Strategy:
  - Batch B=8 sharded across 8 NeuronCores (data parallel), one b per core.
  - Per core, per output group i: gather 15 row-shifted copies of the G_F=7
    selected images into 4 SBUF tiles so that every matmul runs with a fully
    packed K=128 contraction (tap (f,kh) covered by tile block (f,s) at instr
    row-offset D, kh = s + D):
      T1 = (f0@0,f1@0,f2@0,f3@0)  used at D=0..4   -> 20 taps
      T2 = (f4@0,f4@1,f5@0,f5@1)  used at D in {0,2} -> 8 taps
      T3 = (f6@0,f6@1,f6@2,f6@3)  used at D=0      -> 4 taps
      T4 = (f6@4,f4@4,f5@4)       used at D=0      -> 3 taps (K=96)
  - kw0..3 accumulate into ONE psum block via column-shifted rhs: colset 0
    (kw0,kw1) at col offset 0, colset 1 (kw2,kw3) at col offset 2; psum class
    a holds (kw_a + kw_{a+2}) aligned at out col c (class 0) / c-1 (class 1).
    18 matmuls, N = R*61.
  - kw4: two concurrent M=64 column-tiled accumulation chains (psum halves),
    9 matmuls in 5 serial slots, N = R*60.
  - Compute in bf16 (host casts x/W), fp32 PSUM accumulate; rel err ~2e-3.
  - Combine: 1 ScalarE (bias) + 3 VectorE adds, DMA to DRAM.
"""

import os
import sys

import numpy as np
import ml_dtypes

for _p in ("/opt/trn_rl_repo", "/root/.axon_site/_ro/trn_rl_repo"):
    if os.path.isdir(_p) and _p not in sys.path:
        sys.path.append(_p)

import concourse.bass as bass
import concourse.mybir as mybir
import concourse.tile as tile
from concourse import bacc
from concourse.bass_utils import run_bass_kernel_spmd

BF16 = ml_dtypes.bfloat16

B, C, G_IN = 8, 32, 33
O, G_F, KH, KW = 64, 7, 5, 5
X, Y = 64, 64
G_OUT = 15
XO, YO = X - KH + 1, Y - KW + 1  # 60, 60
RCH = 8  # output rows per chunk (8*61 = 488 <= 512 psum bank)

# Instruction list: (tile_id, delta). Tile block layout: list of (f, s).
TILE_BLOCKS = {
    1: [(0, 0), (1, 0), (2, 0), (3, 0)],
    2: [(4, 0), (4, 1), (5, 0), (5, 1)],
    3: [(6, 0), (6, 1), (6, 2), (6, 3)],
    4: [(6, 4), (4, 4), (5, 4)],
}
# Tile row counts (rows materialized in SBUF per block)
TILE_ROWS = {1: 64, 2: 62, 3: 60, 4: 60}
INSTRS = (
    [(1, d) for d in range(5)]
    + [(2, 0), (2, 2)]
    + [(3, 0)]
    + [(4, 0)]
)  # 9 instrs; last has K=96


def _build_nc(idx, n_groups=G_OUT):
    """Build the single-core Bass program (idx values baked into DMAs)."""
    nc = bacc.Bacc("TRN2", target_bir_lowering=False, debug=False)
    dt = mybir.dt
    xin = nc.dram_tensor("x", [C, G_IN, X, Y], dt.bfloat16, kind="ExternalInput")
    wp_d = nc.dram_tensor("wp", [128, 9, 256], dt.bfloat16, kind="ExternalInput")
    wq_d = nc.dram_tensor("wq", [128, 9, O], dt.bfloat16, kind="ExternalInput")
    bias_d = nc.dram_tensor("bias", [O, 1], dt.float32, kind="ExternalInput")
    out_d = nc.dram_tensor("out", [O, G_OUT, XO, YO], dt.float32, kind="ExternalOutput")

    rchunks = [(r0, min(RCH, XO - r0)) for r0 in range(0, XO, RCH)]

    with tile.TileContext(nc) as tc:
        with (
            tc.tile_pool(name="wpool", bufs=1) as wpool,
            tc.tile_pool(name="xpool", bufs=3) as xpool,
            tc.tile_pool(name="tpool", bufs=3) as tpool,
            tc.tile_pool(name="opool", bufs=4) as opool,
            tc.tile_pool(name="psum", bufs=4, space="PSUM") as pp,
            tc.tile_pool(name="psum2", bufs=4, space="PSUM") as pp2,
        ):
            wp = wpool.tile([128, 9, 256], dt.bfloat16, tag="wp")
            wq = wpool.tile([128, 9, O], dt.bfloat16, tag="wq")
            bias_sb = wpool.tile([O, 1], dt.float32, tag="bias")
            # weights on the Activation DGE: parallel with x gathers (sync
            # DGE). Split wp so the T1-instr weights (q=0..4) land first —
            # the head's first matmuls need only those plus t1.
            nc.scalar.dma_start(wp[:, 0:5, :], wp_d[:, 0:5, :])
            nc.scalar.dma_start(wp[:, 5:9, :], wp_d[:, 5:9, :])
            nc.scalar.dma_start(wq[:, :, :], wq_d[:, :, :])
            nc.scalar.dma_start(bias_sb[:, :], bias_d[:, :])

            for i in range(n_groups):
                # Gather the 15 row-shifted image copies into 4 tiles.
                # t4 first: the tile scheduler models DMA arrival from issue
                # order; a late-modeled t4 makes it defer T4 matmuls, which
                # splits the M64 blocks and adds PE mode-switch stalls.
                tiles = {}
                for tid in (1, 2, 3, 4) if i == 0 else (4, 1, 2, 3):
                    nr = TILE_ROWS[tid]
                    blocks = TILE_BLOCKS[tid]
                    npart = 32 * len(blocks)
                    t = xpool.tile([npart, nr, Y], dt.bfloat16, tag=f"t{tid}")
                    tiles[tid] = t
                    for b, (f, s) in enumerate(blocks):
                        g = int(idx[i, f])
                        nc.sync.dma_start(
                            t[b * 32 : (b + 1) * 32, :, :],
                            xin[:, g, s : s + nr, :],
                        )

                # kw0..3: 18 matmuls per rchunk accumulating into pP.
                # colset cs reads x cols 2cs..2cs+60; lhsT cols
                # [cs*128 + a*64 + o] hold w[.., kw=2cs+a].
                def mm_p(pP, r0, R, cs, q, start, stop):
                    tid, d = INSTRS[q]
                    xt = tiles[tid]
                    Kq = 32 * len(TILE_BLOCKS[tid])
                    nc.tensor.matmul(
                        pP[:, 0:R, :],
                        wp[0:Kq, q, cs * 128 : cs * 128 + 128],
                        xt[0:Kq, r0 + d : r0 + d + R, 2 * cs : 2 * cs + 61],
                        start=start,
                        stop=stop,
                    )

                # kw4: two concurrent M=64 column-tiled chains.
                def mm_q2(p2, r0, R, q, half, start, stop):
                    tid, d = INSTRS[q]
                    xt = tiles[tid]
                    Kq = 32 * len(TILE_BLOCKS[tid])
                    nc.tensor.matmul(
                        p2[half * 64 : half * 64 + 64, 0:R, :],
                        wq[0:Kq, q, :],
                        xt[0:Kq, r0 + d : r0 + d + R, 4 : 4 + 60],
                        start=start,
                        stop=stop,
                    )

                # Process rchunks in waves of 4: all M128 pP work, then all
                # M64 kw4 work — quarters PE full/column-tiled mode switches.
                for w0 in range(0, len(rchunks), 4):
                    wave = rchunks[w0 : w0 + 4]
                    pPs, p2s = [], []
                    for r0, R in wave:
                        pPs.append(
                            pp.tile([128, RCH, 61], dt.float32, tag="pP", name="pP")
                        )
                        p2s.append(
                            pp2.tile([128, RCH, 60], dt.float32, tag="p2", name="p2")
                        )
                    if i == 0 and w0 == 0:
                        # head: run every rchunk's T1-only prefix first —
                        # those need just t1 + the first wp slice, giving
                        # the PE a runway while t2..t4 stream in.
                        for (r0, R), pP in zip(wave, pPs):
                            for q in range(5):
                                mm_p(pP, r0, R, 0, q, start=(q == 0), stop=False)
                        for (r0, R), pP in zip(wave, pPs):
                            for q in range(5, 9):
                                mm_p(pP, r0, R, 0, q, start=False, stop=False)
                            for q in range(9):
                                mm_p(pP, r0, R, 1, q, start=False, stop=(q == 8))
                    else:
                        for (r0, R), pP in zip(wave, pPs):
                            for cs in range(2):
                                for q in range(9):
                                    mm_p(
                                        pP, r0, R, cs, q,
                                        start=(cs == 0 and q == 0),
                                        stop=(cs == 1 and q == 8),
                                    )
                    # kw4 column-tile pairs must be CROSS-tile (same-tile
                    # concurrent streams contend on SBUF reads): h0 runs
                    # T1@0..4, h1 runs T2/T3/T4, interleaved pairwise.
                    for (r0, R), p2 in zip(wave, p2s):
                        for k, (q, start, stop) in enumerate(
                            [
                                (0, True, False),   # T1@0  h0 start
                                (5, True, False),   # T2@0  h1 start
                                (1, False, False),  # T1@1  h0
                                (6, False, False),  # T2@2  h1
                                (2, False, False),  # T1@2  h0
                                (7, False, False),  # T3@0  h1
                                (3, False, False),  # T1@3  h0
                                (8, False, True),   # T4@0  h1 stop
                                (4, False, True),   # T1@4  h0 stop
                            ]
                        ):
                            mm_q2(p2, r0, R, q, half=k % 2, start=start, stop=stop)

                    # Combine: at most ONE PSUM operand per instruction.
                    # out[c] = bias + pP[0:64,:,c] + pP[64:,:,c+1]
                    #        + p2[0:64,:,c] + p2[64:,:,c]
                    for (r0, R), pP, p2 in zip(wave, pPs, p2s):
                        t = tpool.tile([O, RCH, 60], dt.float32, tag="t")
                        ot = opool.tile([O, RCH, 60], dt.float32, tag="out")
                        nc.scalar.add(
                            t[:, 0:R, :], pP[0:64, 0:R, 0:60], bias_sb[:, 0:1]
                        )
                        nc.vector.tensor_add(
                            t[:, 0:R, :], t[:, 0:R, :], pP[64:128, 0:R, 1:61]
                        )
                        nc.vector.tensor_add(
                            t[:, 0:R, :], t[:, 0:R, :], p2[0:64, 0:R, :]
                        )
                        nc.vector.tensor_add(
                            ot[:, 0:R, :], t[:, 0:R, :], p2[64:128, 0:R, :]
                        )
                        # out via the Activation-engine DGE: separate DMA
                        # queues from the x gathers, so gathers never sit
                        # behind compute-gated output writes in a queue FIFO.
                        nc.scalar.dma_start(
                            out_d[:, i, r0 : r0 + R, :], ot[:, 0:R, :]
                        )
    nc.compile()
    return nc


def _prep_weights(weight, bias):
    """Host-side lhsT weight layout for the 9-instruction schedule."""
    w = np.asarray(weight).astype(np.float32)  # [O, C, G_F, KH, KW]
    wp = np.zeros((128, 9, 256), dtype=np.float32)
    wq = np.zeros((128, 9, O), dtype=np.float32)
    for q, (tid, d) in enumerate(INSTRS):
        for b, (f, s) in enumerate(TILE_BLOCKS[tid]):
            kh = s + d
            rows = slice(b * 32, (b + 1) * 32)
            for cs in range(2):
                for a in range(2):
                    kw = 2 * cs + a
                    col = cs * 128 + a * 64
                    # wp[row=c, q, col+o] = w[o, c, f, kh, kw]
                    wp[rows, q, col : col + O] = w[:, :, f, kh, kw].T
            wq[rows, q, :] = w[:, :, f, kh, 4].T
    b2 = np.ascontiguousarray(np.asarray(bias).astype(np.float32).reshape(O, 1))
    return wp.astype(BF16), wq.astype(BF16), b2


def _prep_inputs(x, weight, bias, idx):
    x16 = np.asarray(x).astype(BF16)  # [B, C, G_IN, X, Y]
    wp, wq, b2 = _prep_weights(weight, bias)
    in_maps = []
    for b in range(B):
        in_maps.append(
            {
                "x": np.ascontiguousarray(x16[b]),
                "wp": wp,
                "wq": wq,
                "bias": b2,
            }
        )
    return in_maps


def run(x, weight, bias, idx, trace=False):
    idx = np.asarray(idx).astype(np.int64)
    assert idx.shape == (G_OUT, G_F) and idx.min() >= 0 and idx.max() < G_IN
    nc = _build_nc(idx)
    in_maps = _prep_inputs(x, weight, bias, idx)
    res = run_bass_kernel_spmd(nc, in_maps, list(range(B)), trace=trace)
    out = np.stack([res.results[b]["out"] for b in range(B)]).astype(np.float32)
    return out, res


def kernel(x, weight, bias, idx):
    out, _ = run(x, weight, bias, idx, trace=False)
    return out


# revision 30
# speedup vs baseline: 1.0077x; 1.0011x over previous
"""Trainium2 Bass kernel for nn_GroupLocalSL2 (grouped gather + conv).

out[b,o,i,xo,yo] = sum_{c,f,kh,kw} x[b,c,idx[i,f],xo+kh,yo+kw] * W[o,c,f,kh,kw] + bias[o]

Strategy:
  - Batch B=8 sharded across 8 NeuronCores (data parallel), one b per core.
  - Per core, per output group i: gather 15 row-shifted copies of the G_F=7
    selected images into 4 SBUF tiles so that every matmul runs with a fully
    packed K=128 contraction (tap (f,kh) covered by tile block (f,s) at instr
    row-offset D, kh = s + D):
      T1 = (f0@0,f1@0,f2@0,f3@0)  used at D=0..4   -> 20 taps
      T2 = (f4@0,f4@1,f5@0,f5@1)  used at D in {0,2} -> 8 taps
      T3 = (f6@0,f6@1,f6@2,f6@3)  used at D=0      -> 4 taps
      T4 = (f6@4,f4@4,f5@4)       used at D=0      -> 3 taps (K=96)
  - kw0..3 accumulate into ONE psum block via column-shifted rhs: colset 0
    (kw0,kw1) at col offset 0, colset 1 (kw2,kw3) at col offset 2; psum class
    a holds (kw_a + kw_{a+2}) aligned at out col c (class 0) / c-1 (class 1).
    18 matmuls, N = R*61.
  - kw4: two concurrent M=64 column-tiled accumulation chains (psum halves),
    9 matmuls in 5 serial slots, N = R*60.
  - Compute in bf16 (host casts x/W), fp32 PSUM accumulate; rel err ~2e-3.
  - Combine: 1 ScalarE (bias) + 3 VectorE adds, DMA to DRAM.
"""

import os
import sys

import numpy as np
import ml_dtypes

for _p in ("/opt/trn_rl_repo", "/root/.axon_site/_ro/trn_rl_repo"):
    if os.path.isdir(_p) and _p not in sys.path:
        sys.path.append(_p)

import concourse.bass as bass
import concourse.mybir as mybir
import concourse.tile as tile
from concourse import bacc
from concourse.bass_utils import run_bass_kernel_spmd

BF16 = ml_dtypes.bfloat16

B, C, G_IN = 8, 32, 33
O, G_F, KH, KW = 64, 7, 5, 5
X, Y = 64, 64
G_OUT = 15
XO, YO = X - KH + 1, Y - KW + 1  # 60, 60
RCH = 8  # output rows per chunk (8*61 = 488 <= 512 psum bank)

# Instruction list: (tile_id, delta). Tile block layout: list of (f, s).
TILE_BLOCKS = {
    1: [(0, 0), (1, 0), (2, 0), (3, 0)],
    2: [(4, 0), (4, 1), (5, 0), (5, 1)],
    3: [(6, 0), (6, 1), (6, 2), (6, 3)],
    4: [(6, 4), (4, 4), (5, 4)],
}
# Tile row counts (rows materialized in SBUF per block)
TILE_ROWS = {1: 64, 2: 62, 3: 60, 4: 60}
INSTRS = (
    [(1, d) for d in range(5)]
    + [(2, 0), (2, 2)]
    + [(3, 0)]
    + [(4, 0)]
)  # 9 instrs; last has K=96


def _build_nc(idx, n_groups=G_OUT):
    """Build the single-core Bass program (idx values baked into DMAs)."""
    nc = bacc.Bacc("TRN2", target_bir_lowering=False, debug=False)
    dt = mybir.dt
    xin = nc.dram_tensor("x", [C, G_IN, X, Y], dt.bfloat16, kind="ExternalInput")
    wp_d = nc.dram_tensor("wp", [128, 9, 256], dt.bfloat16, kind="ExternalInput")
    wq_d = nc.dram_tensor("wq", [128, 9, O], dt.bfloat16, kind="ExternalInput")
    bias_d = nc.dram_tensor("bias", [O, 1], dt.float32, kind="ExternalInput")
    out_d = nc.dram_tensor("out", [O, G_OUT, XO, YO], dt.float32, kind="ExternalOutput")

    rchunks = [(r0, min(RCH, XO - r0)) for r0 in range(0, XO, RCH)]

    with tile.TileContext(nc) as tc:
        with (
            tc.tile_pool(name="wpool", bufs=1) as wpool,
            tc.tile_pool(name="xpool", bufs=3) as xpool,
            tc.tile_pool(name="tpool", bufs=3) as tpool,
            tc.tile_pool(name="opool", bufs=4) as opool,
            tc.tile_pool(name="psum", bufs=4, space="PSUM") as pp,
            tc.tile_pool(name="psum2", bufs=4, space="PSUM") as pp2,
        ):
            # PE clock warm-up: first matmuls after idle run at the cold
            # ~1.2GHz rate. K=1 dummies don't trigger the ramp; use full
            # K=128 rows on memset data, ending before the real deps land.
            warm = wpool.tile([128, 512], dt.bfloat16, tag="warm")
            pwarm = pp.tile([128, RCH, 61], dt.float32, tag="pP", name="pwarm")
            nc.gpsimd.memset(warm[:, :], 0.0)
            for _ in range(16):
                nc.tensor.matmul(
                    pwarm[0:1, :, :], warm[0:128, 0:1], warm[0:128, 0:488],
                    start=True, stop=True,
                )

            wp = wpool.tile([128, 9, 256], dt.bfloat16, tag="wp")
            wq = wpool.tile([128, 9, O], dt.bfloat16, tag="wq")
            bias_sb = wpool.tile([O, 1], dt.float32, tag="bias")
            # weights on the Activation DGE: parallel with x gathers (sync
            # DGE). Split wp so the T1-instr weights (q=0..4) land first —
            # the head's first matmuls need only those plus t1.
            nc.scalar.dma_start(wp[:, 0:5, :], wp_d[:, 0:5, :])
            nc.scalar.dma_start(wp[:, 5:9, :], wp_d[:, 5:9, :])
            nc.scalar.dma_start(wq[:, :, :], wq_d[:, :, :])
            nc.scalar.dma_start(bias_sb[:, :], bias_d[:, :])

            for i in range(n_groups):
                # Gather the 15 row-shifted image copies into 4 tiles.
                # t4 first: the tile scheduler models DMA arrival from issue
                # order; a late-modeled t4 makes it defer T4 matmuls, which
                # splits the M64 blocks and adds PE mode-switch stalls.
                tiles = {}
                for tid in (1, 2, 3, 4) if i == 0 else (4, 1, 2, 3):
                    nr = TILE_ROWS[tid]
                    blocks = TILE_BLOCKS[tid]
                    npart = 32 * len(blocks)
                    t = xpool.tile([npart, nr, Y], dt.bfloat16, tag=f"t{tid}")
                    tiles[tid] = t
                    for b, (f, s) in enumerate(blocks):
                        g = int(idx[i, f])
                        nc.sync.dma_start(
                            t[b * 32 : (b + 1) * 32, :, :],
                            xin[:, g, s : s + nr, :],
                        )

                # kw0..3: 18 matmuls per rchunk accumulating into pP.
                # colset cs reads x cols 2cs..2cs+60; lhsT cols
                # [cs*128 + a*64 + o] hold w[.., kw=2cs+a].
                def mm_p(pP, r0, R, cs, q, start, stop):
                    tid, d = INSTRS[q]
                    xt = tiles[tid]
                    Kq = 32 * len(TILE_BLOCKS[tid])
                    nc.tensor.matmul(
                        pP[:, 0:R, :],
                        wp[0:Kq, q, cs * 128 : cs * 128 + 128],
                        xt[0:Kq, r0 + d : r0 + d + R, 2 * cs : 2 * cs + 61],
                        start=start,
                        stop=stop,
                    )

                # kw4: two concurrent M=64 column-tiled chains.
                def mm_q2(p2, r0, R, q, half, start, stop):
                    tid, d = INSTRS[q]
                    xt = tiles[tid]
                    Kq = 32 * len(TILE_BLOCKS[tid])
                    nc.tensor.matmul(
                        p2[half * 64 : half * 64 + 64, 0:R, :],
                        wq[0:Kq, q, :],
                        xt[0:Kq, r0 + d : r0 + d + R, 4 : 4 + 60],
                        start=start,
                        stop=stop,
                    )

                # Process rchunks in waves of 4: all M128 pP work, then all
                # M64 kw4 work — quarters PE full/column-tiled mode switches.
                for w0 in range(0, len(rchunks), 4):
                    wave = rchunks[w0 : w0 + 4]
                    pPs, p2s = [], []
                    for r0, R in wave:
                        pPs.append(
                            pp.tile([128, RCH, 61], dt.float32, tag="pP", name="pP")
                        )
                        p2s.append(
                            pp2.tile([128, RCH, 60], dt.float32, tag="p2", name="p2")
                        )
                    if i == 0 and w0 == 0:
                        # head: run every rchunk's T1-only prefix first —
                        # those need just t1 + the first wp slice, giving
                        # the PE a runway while t2..t4 stream in.
                        for (r0, R), pP in zip(wave, pPs):
                            for q in range(5):
                                mm_p(pP, r0, R, 0, q, start=(q == 0), stop=False)
                        for (r0, R), pP in zip(wave, pPs):
                            for q in range(5, 9):
                                mm_p(pP, r0, R, 0, q, start=False, stop=False)
                            for q in range(9):
                                mm_p(pP, r0, R, 1, q, start=False, stop=(q == 8))
                    else:
                        for (r0, R), pP in zip(wave, pPs):
                            for cs in range(2):
                                for q in range(9):
                                    mm_p(
                                        pP, r0, R, cs, q,
                                        start=(cs == 0 and q == 0),
                                        stop=(cs == 1 and q == 8),
                                    )
                    # kw4 column-tile pairs must be CROSS-tile (same-tile
                    # concurrent streams contend on SBUF reads): h0 runs
                    # T1@0..4, h1 runs T2/T3/T4, interleaved pairwise.
                    for (r0, R), p2 in zip(wave, p2s):
                        for k, (q, start, stop) in enumerate(
                            [
                                (0, True, False),   # T1@0  h0 start
                                (5, True, False),   # T2@0  h1 start
                                (1, False, False),  # T1@1  h0
                                (6, False, False),  # T2@2  h1
                                (2, False, False),  # T1@2  h0
                                (7, False, False),  # T3@0  h1
                                (3, False, False),  # T1@3  h0
                                (8, False, True),   # T4@0  h1 stop
                                (4, False, True),   # T1@4  h0 stop
                            ]
                        ):
                            mm_q2(p2, r0, R, q, half=k % 2, start=start, stop=stop)

                    # Combine: at most ONE PSUM operand per instruction.
                    # out[c] = bias + pP[0:64,:,c] + pP[64:,:,c+1]
                    #        + p2[0:64,:,c] + p2[64:,:,c]
                    for (r0, R), pP, p2 in zip(wave, pPs, p2s):
                        t = tpool.tile([O, RCH, 60], dt.float32, tag="t")
                        ot = opool.tile([O, RCH, 60], dt.float32, tag="out")
                        nc.scalar.add(
                            t[:, 0:R, :], pP[0:64, 0:R, 0:60], bias_sb[:, 0:1]
                        )
                        nc.vector.tensor_add(
                            t[:, 0:R, :], t[:, 0:R, :], pP[64:128, 0:R, 1:61]
                        )
                        nc.vector.tensor_add(
                            t[:, 0:R, :], t[:, 0:R, :], p2[0:64, 0:R, :]
                        )
                        nc.vector.tensor_add(
                            ot[:, 0:R, :], t[:, 0:R, :], p2[64:128, 0:R, :]
                        )
                        # out via the Activation-engine DGE: separate DMA
                        # queues from the x gathers, so gathers never sit
                        # behind compute-gated output writes in a queue FIFO.
                        nc.scalar.dma_start(
                            out_d[:, i, r0 : r0 + R, :], ot[:, 0:R, :]
                        )
    nc.compile()
    return nc


def _prep_weights(weight, bias):
    """Host-side lhsT weight layout for the 9-instruction schedule."""
    w = np.asarray(weight).astype(np.float32)  # [O, C, G_F, KH, KW]
    wp = np.zeros((128, 9, 256), dtype=np.float32)
    wq = np.zeros((128, 9, O), dtype=np.float32)
    for q, (tid, d) in enumerate(INSTRS):
        for b, (f, s) in enumerate(TILE_BLOCKS[tid]):
            kh = s + d
            rows = slice(b * 32, (b + 1) * 32)
            for cs in range(2):
                for a in range(2):
                    kw = 2 * cs + a
                    col = cs * 128 + a * 64
                    # wp[row=c, q, col+o] = w[o, c, f, kh, kw]
                    wp[rows, q, col : col + O] = w[:, :, f, kh, kw].T
            wq[rows, q, :] = w[:, :, f, kh, 4].T
    b2 = np.ascontiguousarray(np.asarray(bias).astype(np.float32).reshape(O, 1))
    return wp.astype(BF16), wq.astype(BF16), b2


def _prep_inputs(x, weight, bias, idx):
    x16 = np.asarray(x).astype(BF16)  # [B, C, G_IN, X, Y]
    wp, wq, b2 = _prep_weights(weight, bias)
    in_maps = []
    for b in range(B):
        in_maps.append(
            {
                "x": np.ascontiguousarray(x16[b]),
                "wp": wp,
                "wq": wq,
                "bias": b2,
            }
        )
    return in_maps


def run(x, weight, bias, idx, trace=False):
    idx = np.asarray(idx).astype(np.int64)
    assert idx.shape == (G_OUT, G_F) and idx.min() >= 0 and idx.max() < G_IN
    nc = _build_nc(idx)
    in_maps = _prep_inputs(x, weight, bias, idx)
    res = run_bass_kernel_spmd(nc, in_maps, list(range(B)), trace=trace)
    out = np.stack([res.results[b]["out"] for b in range(B)]).astype(np.float32)
    return out, res


def kernel(x, weight, bias, idx):
    out, _ = run(x, weight, bias, idx, trace=False)
    return out
